# revision 6
# baseline (speedup 1.0000x reference)
"""MinibatchDiscrimination TRN2 kernel (v3).

x: [512, 1024] f32, T: [1024, 1024] f32.
M = (x @ T).reshape(512, 64, 16); l1[i,j,k] = sum_d |M[i,kd]-M[j,kd]|
out[i,k] = sum_j exp(-l1[i,j,k]) - 1.

Sharding: batch rows split across 8 cores (64 each), no collectives. Each
core's copy of x^T has its j-columns rolled so its own 64 rows sit at local
columns 0..63.

Symmetric pair coverage: core-local row i computes the j-window
[i+1, i+257) (pair distance d in [1, 256]). Every unordered pair {a, a+d}
with d in [1, 255] is computed exactly once (accumulated to row a via the
exp's accum_out and to row a+d via a transposed column accumulator);
d = 256 pairs are computed twice, row-side only. The diagonal is never
computed, so no -1 correction is needed.

|z| handling per kd-tile: tiles 0..6 run on DVE as relu(z) = (z sub col)
max 0 in bf16 (4x DVE mode) with the 2*relu identity
  sum_d |z_d| = 2*sum_d relu(z_d) - (G_j - G_i),  G_j = sum_d M[j, kd];
the -G_j term is injected into the l1 PSUM via an I64 x Gsneg matmul
(f32r, exact) and +G_i rides the exp bias. Tile 7 runs on ACT as a direct
Abs activation (bias=col, scale=-1) and needs no correction — its Gsneg
rows are zero (the ISA rejects abs_max on DVE tensor_scalar, hence the
split). S entries are 2.0 for tiles 0..6 and 1.0 for tile 7.

Phase 2 is bf16 (M^T tiles, relu/abs tiles, E tiles; l1 accumulates fp32
in PSUM). Rows are processed in PAIRS sharing [128, 512] tiles and one
[64, 512] l1 PSUM bank: 8 S-matmuls of FD=512 per pair. The transposed
column accumulation runs on the PE (per row one I64 matmul of E[:, 0:255]
into a persistent PSUM O_col bank, zeroed once by a rank-1 zero matmul).

Phase 1 is f-major with 7 concurrent PSUM banks (t=0..6 accumulate as
x/T chunks stream in; t=7 in a short second pass) to minimise the PE
tail after the DMA window.

Engine budget per pair (@2.4GHz PE): PE 8x512 + 2x256 + 2x255 = 5118 cyc
(~2.13us), DVE 14 relu x ~127ns = 1.78us, ACT 2 abs + 2 exp = ~1.97us.
PE stays the busiest engine so its HAM clock gate stays at 8/8 (2.4GHz).
"""

import contextlib

import numpy as np
import ml_dtypes

import concourse.bass as bass
import concourse.tile as tile
from concourse import mybir
from concourse import bass_utils

B = 512
F = 1024
KD = 1024  # = NUM_KERNELS(64) * KERNEL_DIM(16)
NK = 64
N_CORES = 8
NI = B // N_CORES  # local rows per core
NT = KD // 128  # kd tiles
NF = F // 128  # f chunks
W = 256  # j-window width per row
JL = NI + W  # used local-j extent
NP = NI // 2  # row pairs
N_DVE = 7  # tiles 0..N_DVE-1 on DVE (relu+G), rest on ACT (direct Abs)

_FP32 = mybir.dt.float32
_F32R = mybir.dt.float32r
_BF16 = mybir.dt.bfloat16


def _split_all_waits(nc):
    """walrus in this env encodes at most 1 sync wait per instruction: hoist
    extra waits onto same-engine NOPs inserted just before the instruction.
    Safe because waits are AND-ed stall conditions on the engine's sequencer
    and semaphores are monotonic."""
    count = 0
    for fn in nc.m.functions:
        for bb in fn.blocks:
            insts = list(bb.instructions)
            new = []
            changed = False
            for inst in insts:
                si = getattr(inst, "sync_info", None)
                waits = list(si.on_wait) if (si is not None and si.on_wait) else []
                if len(waits) > 1:
                    for w in waits[:-1]:
                        nop = mybir.InstNoOp(name=f"NOPW-{count}", ins=[], outs=[])
                        count += 1
                        nop.engine = inst.engine
                        nop.sync_info = mybir.SyncInfo(on_wait=[w], on_update=[])
                        nc.register_instruction(nop, overwrite=True)
                        new.append(nop)
                    si.on_wait = [waits[-1]]
                    changed = True
                new.append(inst)
            if changed:
                bb.instructions[:] = new


def _patch_drain_wait_limit():
    if getattr(tile.TileContext, "_wait_split_patched", False):
        return
    orig = tile.TileContext.schedule_and_allocate

    def schedule_and_allocate(self, *a, **k):
        r = orig(self, *a, **k)
        _split_all_waits(self.nc)
        return r

    tile.TileContext.schedule_and_allocate = schedule_and_allocate
    tile.TileContext._wait_split_patched = True


def build_s_matrices():
    """Per-kd-tile d-sum matrices (tile t holds k in [8t, 8t+8)):
    S[p, t*NK + k] = w_t iff k == 8t + p//16, w_t = 2.0 for DVE (relu)
    tiles, 1.0 for ACT (abs) tiles. Sg builds Gsneg = -G for DVE tiles
    (zero rows for ACT tiles). I64 = identity."""
    S = np.zeros((128, NT * NK), dtype=ml_dtypes.bfloat16)
    Sg = np.zeros((128, NT * NK), dtype=ml_dtypes.bfloat16)
    for t in range(NT):
        for p in range(128):
            k = t * NK + 8 * t + p // 16
            if t < N_DVE:
                S[p, k] = 2.0
                Sg[p, k] = -1.0
            else:
                S[p, k] = 1.0
    I64_bf = np.eye(NK, dtype=ml_dtypes.bfloat16)
    I64_f = np.eye(NK, dtype=np.float32)
    return S, Sg, I64_bf, I64_f


def build_program(repeat: int = 1):
    _patch_drain_wait_limit()
    nc = bass.Bass(
        "TRN2", target_bir_lowering=False, debug=False, num_devices=N_CORES
    )
    xT_d = nc.dram_tensor("xT", [F, JL], _FP32, kind="ExternalInput").ap()
    T_d = nc.dram_tensor("T", [F, KD], _FP32, kind="ExternalInput").ap()
    S_d = nc.dram_tensor("S", [128, NT * NK], _BF16, kind="ExternalInput").ap()
    Sg_d = nc.dram_tensor("Sg", [128, NT * NK], _BF16, kind="ExternalInput").ap()
    I64b_d = nc.dram_tensor("I64b", [NK, NK], _BF16, kind="ExternalInput").ap()
    I64f_d = nc.dram_tensor("I64f", [NK, NK], _FP32, kind="ExternalInput").ap()
    orow_d = nc.dram_tensor("orow", [NK, NI], _FP32, kind="ExternalOutput").ap()
    ocol_d = nc.dram_tensor("ocol", [NK, JL], _FP32, kind="ExternalOutput").ap()

    AF = mybir.ActivationFunctionType
    AO = mybir.AluOpType

    with tile.TileContext(nc) as tc:
        with (
            tc.tile_pool(name="stage", bufs=3) as stage,
            tc.tile_pool(name="tr", bufs=NF) as tr_pool,
            tc.tile_pool(name="xr", bufs=NF) as xr_pool,
            tc.tile_pool(name="mt", bufs=NT) as mt_pool,
            tc.tile_pool(name="ssb", bufs=1) as s_pool,
            tc.tile_pool(name="ab", bufs=16) as ab_pool,
            tc.tile_pool(name="ep", bufs=4) as e_pool,
            tc.tile_pool(name="op", bufs=1) as o_pool,
            tc.tile_pool(name="pp", bufs=7, space="PSUM") as psum_p,
            tc.tile_pool(name="pcol", bufs=1, space="PSUM") as psum_col,
            tc.For_i(0, repeat, 1) if repeat > 1 else contextlib.nullcontext(),
        ):
            # ---- loads + f32r rounding (phase-1 inputs stay fp32) ----
            T_r = []
            x_r = []
            for f in range(NF):
                st = stage.tile([128, JL], _FP32, tag="xstage")
                nc.sync.dma_start(out=st, in_=xT_d[f * 128 : (f + 1) * 128, :])
                xr = xr_pool.tile([128, JL], _F32R, tag="xr")
                nc.vector.tensor_copy(xr, st)
                x_r.append(xr)
                st = stage.tile([128, KD], _FP32, tag="stage")
                nc.sync.dma_start(out=st, in_=T_d[f * 128 : (f + 1) * 128, :])
                tr = tr_pool.tile([128, KD], _F32R, tag="tr")
                if f % 2 == 0:
                    nc.vector.tensor_copy(tr, st)
                else:
                    nc.scalar.copy(tr, st)
                T_r.append(tr)
            S_sb = s_pool.tile([128, NT * NK], _BF16, tag="s")
            nc.sync.dma_start(out=S_sb, in_=S_d)
            Sg_sb = s_pool.tile([128, NT * NK], _BF16, tag="sg")
            nc.sync.dma_start(out=Sg_sb, in_=Sg_d)
            I64_bf = s_pool.tile([NK, NK], _BF16, tag="i64b")
            nc.sync.dma_start(out=I64_bf, in_=I64b_d)
            st = stage.tile([NK, NK], _FP32, tag="i64stage")
            nc.sync.dma_start(out=st, in_=I64f_d)
            I64_r = s_pool.tile([NK, NK], _F32R, tag="i64r")
            nc.vector.tensor_copy(I64_r, st)
            zrow = s_pool.tile([1, NK], _BF16, tag="z64")
            nc.vector.memset(zrow, 0.0)
            zJL = s_pool.tile([1, JL], _BF16, tag="zjl")
            nc.vector.memset(zJL, 0.0)

            # ---- phase 1: M^T tiles [128 kd, JL j] (bf16) + fp32 cols ----
            # f-major so the PE accumulates as chunks arrive; t=0..6 live in
            # 7 PSUM banks, t=7 runs as a short second pass reusing a bank.
            col_f32 = s_pool.tile([128, NT * NI], _FP32, tag="colf")
            mt = [None] * NT

            def finish_tile(t, pm):
                m = mt_pool.tile([128, JL], _BF16, tag="mt")
                if t % 2 == 0:
                    nc.vector.tensor_copy(m, pm)
                    nc.scalar.copy(col_f32[:, t * NI : (t + 1) * NI], pm[:, 0:NI])
                else:
                    nc.scalar.copy(m, pm)
                    nc.vector.tensor_copy(
                        col_f32[:, t * NI : (t + 1) * NI], pm[:, 0:NI]
                    )
                mt[t] = m

            pmA = [
                psum_p.tile([128, JL], _FP32, tag="pp", name=f"pmA{t}")
                for t in range(7)
            ]
            for f in range(NF):
                for t in range(7):
                    nc.tensor.matmul(
                        pmA[t],
                        lhsT=T_r[f][:, t * 128 : (t + 1) * 128],
                        rhs=x_r[f],
                        start=(f == 0),
                        stop=(f == NF - 1),
                    )
            for t in range(7):
                finish_tile(t, pmA[t])
            pm7 = psum_p.tile([128, JL], _FP32, tag="pp")
            for f in range(NF):
                nc.tensor.matmul(
                    pm7,
                    lhsT=T_r[f][:, 7 * 128 : 8 * 128],
                    rhs=x_r[f],
                    start=(f == 0),
                    stop=(f == NF - 1),
                )
            finish_tile(7, pm7)

            # ---- phase 1.5: Gsneg[k, j] = -G[k, j] for DVE-tile k ----
            pg = psum_p.tile([NK, JL], _FP32, tag="pp")
            for t in range(NT):
                nc.tensor.matmul(
                    pg,
                    lhsT=Sg_sb[:, t * NK : (t + 1) * NK],
                    rhs=mt[t],
                    start=(t == 0),
                    stop=(t == NT - 1),
                )
            Gs_r = s_pool.tile([NK, JL], _F32R, tag="gsr")
            nc.vector.tensor_copy(Gs_r, pg)

            # ---- phase 2 ----
            O_row = o_pool.tile([NK, NI], _FP32, tag="orow")
            O_colp = psum_col.tile([NK, JL], _FP32, tag="ocolp")
            # zero the whole column accumulator (rank-1 zero matmul opens
            # the accumulation group over the full width)
            nc.tensor.matmul(O_colp, lhsT=zrow, rhs=zJL, start=True, stop=False)

            for p in range(NP):
                i0, i1 = 2 * p, 2 * p + 1
                abt = []
                for t in range(NT):
                    ab = ab_pool.tile([128, 2 * W], _BF16, tag="ab")
                    for h, i in ((0, i0), (1, i1)):
                        dst = ab[:, h * W : (h + 1) * W]
                        src = mt[t][:, i + 1 : i + 1 + W]
                        col = col_f32[:, t * NI + i : t * NI + i + 1]
                        if t < N_DVE:
                            nc.vector.tensor_scalar(
                                dst, src, col, 0.0, op0=AO.subtract, op1=AO.max
                            )
                        else:
                            nc.scalar.activation(
                                dst, src, AF.Abs, bias=col, scale=-1.0
                            )
                    abt.append(ab)
                l1 = psum_p.tile([NK, 2 * W], _FP32, tag="pp")
                for t in range(NT):
                    nc.tensor.matmul(
                        l1,
                        lhsT=S_sb[:, t * NK : (t + 1) * NK],
                        rhs=abt[t],
                        start=(t == 0),
                        stop=False,
                    )
                for h, i in ((0, i0), (1, i1)):
                    nc.tensor.matmul(
                        l1[:, h * W : (h + 1) * W],
                        lhsT=I64_r,
                        rhs=Gs_r[:, i + 1 : i + 1 + W],
                        start=False,
                        stop=(h == 1),
                    )
                E = e_pool.tile([NK, 2 * W], _BF16, tag="ep")
                for h, i in ((0, i0), (1, i1)):
                    nc.scalar.activation(
                        E[:, h * W : (h + 1) * W],
                        l1[:, h * W : (h + 1) * W],
                        AF.Exp,
                        scale=-1.0,
                        bias=Gs_r[:, i : i + 1].bitcast(_FP32),
                        accum_out=O_row[:, i : i + 1],
                    )
                for h, i in ((0, i0), (1, i1)):
                    nc.tensor.matmul(
                        O_colp[:, i + 1 : i + W],
                        lhsT=I64_bf,
                        rhs=E[:, h * W : h * W + W - 1],
                        start=False,
                        stop=(p == NP - 1 and h == 1),
                    )

            O_col = o_pool.tile([NK, JL], _FP32, tag="ocol")
            nc.scalar.copy(O_col, O_colp)
            nc.sync.dma_start(out=orow_d, in_=O_row)
            nc.sync.dma_start(out=ocol_d, in_=O_col)
    return nc


_CACHED = {}


def _get_program(repeat: int = 1):
    key = f"nc{repeat}"
    if key not in _CACHED:
        _CACHED[key] = build_program(repeat)
        _CACHED["S"] = build_s_matrices()
    return _CACHED[key], _CACHED["S"]


def make_in_maps(x: np.ndarray, T: np.ndarray, S, Sg, I64b, I64f):
    xT = np.ascontiguousarray(x.T.astype(np.float32, copy=False))
    T_arr = np.ascontiguousarray(T.astype(np.float32, copy=False))
    in_maps = []
    for c in range(N_CORES):
        xTc = np.ascontiguousarray(np.roll(xT, -NI * c, axis=1)[:, :JL])
        in_maps.append(
            {"xT": xTc, "T": T_arr, "S": S, "Sg": Sg, "I64b": I64b, "I64f": I64f}
        )
    return in_maps


def assemble(results) -> np.ndarray:
    out = np.zeros((B, NK), dtype=np.float64)
    for c in range(N_CORES):
        R = results[c]["orow"]  # [k, i_local]
        C = results[c]["ocol"]  # [k, j_local]
        out[NI * c : NI * (c + 1), :] += R.T
        Cfull = np.zeros((B, NK), dtype=np.float64)
        Cfull[:JL] = C.T
        out += np.roll(Cfull, NI * c, axis=0)
    return out.astype(np.float32)


def run(
    x: np.ndarray,
    T: np.ndarray,
    trace: bool = False,
    repeat: int = 1,
    tmpdir: str | None = None,
):
    nc, (S, Sg, I64b, I64f) = _get_program(repeat)
    in_maps = make_in_maps(x, T, S, Sg, I64b, I64f)
    res = bass_utils.run_bass_kernel_spmd(
        nc, in_maps, core_ids=list(range(N_CORES)), trace=trace, tmpdir=tmpdir
    )
    return assemble(res.results), res


def kernel(x: np.ndarray, T: np.ndarray) -> np.ndarray:
    out, _ = run(x, T)
    return out


# revision 12
# speedup vs baseline: 1.0455x; 1.0455x over previous
"""MinibatchDiscrimination TRN2 kernel (v4).

x: [512, 1024] f32, T: [1024, 1024] f32.
M = (x @ T).reshape(512, 64, 16); l1[i,j,k] = sum_d |M[i,kd]-M[j,kd]|
out[i,k] = sum_j exp(-l1[i,j,k]) - 1.

Sharding: batch rows split across 8 cores (64 each), no collectives. Each
core's copy of x^T has its j-columns rolled so its own 64 rows sit at
local columns 0..63. Symmetric pair coverage as v1-v3: row i covers the
j-window [i+1, i+257); d=1..255 pairs counted once (row-side via exp
accum, col-side via a PSUM column accumulator on the PE), d=256 twice
(row-side only); the diagonal is never computed.

Measured op costs (this silicon): DVE tensor_scalar = 58 + FD/2 cyc
(2x max — no 4x uop for TS, any dtype); ACT activation = ~(init + FD)
cyc at 1.2GHz with init 222 (SBUF) / 172 (PSUM src) + 284ns for the
accumulator read; GPSIMD elementwise is ~4us/tile (useless); PE bf16
matmul = 1 col/cyc at 2.4GHz once the HAM clock gate is warm.

v4 structure:
- Uniform s=+1 relu identity for ALL kd tiles:
    sum_d |z_d| = 2*sum_d relu(z_d) - (G_j - G_i),  G_j = sum_d M[j,kd]
  DVE computes relu via (z sub col) max 0 (tiles 0..5, bf16 in/out,
  ~196ns); ACT computes relu via Relu(in + bias=-col) reading M^T
  directly from resident PSUM banks (tiles 6..7, ~357ns). -G_j is
  injected into the l1 PSUM by I64-style matmuls; +G_i rides exp bias.
- 2-pair k-stacked groups: 4 rows per group. l1 PSUM is [128, 512]:
  partitions 0..63 = pair A's 64 kernels, 64..127 = pair B's. The S
  matmul lhsT is [128, 128] (lo variant fills output rows 0..63 from
  pair A's relu tile, hi variant rows 64..127 from pair B's). One exp
  instruction [128, 256] then covers TWO rows (halves ACT exp+accum
  cost/row); its bias/accum use k-stacked Gb / O_row2 [128, .] built
  with selection matmuls.
- Column accumulation on the PE: rhs = full E [128, 255], lhsT =
  selection matrix Isel_v [128, 64] picking pair v's half; accumulates
  into a persistent PSUM O_col bank (zeroed once by a rank-1 matmul).
- bf16 host-packed loads: T/x are shipped pre-transposed as [128, .]
  bf16 so phase 1 needs no staging or cast; phase-1 matmuls are bf16.
- PE warmup: 12 dummy FD=512 matmuls during the DMA window so the HAM
  clock gate reaches 2.4GHz before phase 1.
- Phase 1 f-major with 6 live PSUM banks (tiles 0..5) as chunks
  stream in; tiles 6..7 in a short pass B stay PSUM-resident for ACT.

Engine budget per pair (measured rates): PE 8x512+2x256+2x255 cyc
~2.3us/pair-equivalent, DVE 12 relu x 196ns = 2.35us, ACT 4 relu x
357 + exp-side 744 = 2.17us.
"""

import contextlib

import numpy as np
import ml_dtypes

import concourse.bass as bass
import concourse.tile as tile
from concourse import mybir
from concourse import bass_utils

B = 512
F = 1024
KD = 1024  # = NUM_KERNELS(64) * KERNEL_DIM(16)
NK = 64
N_CORES = 8
NI = B // N_CORES  # local rows per core
NT = KD // 128  # kd tiles
NF = F // 128  # f chunks
W = 256  # j-window width per row
JL = NI + W  # used local-j extent
NG = NI // 4  # 2-pair groups (4 rows each)
N_DVE = 6  # tiles 0..5 on DVE, 6..7 on ACT (from resident PSUM)

_FP32 = mybir.dt.float32
_F32R = mybir.dt.float32r
_BF16 = mybir.dt.bfloat16


def _split_all_waits(nc):
    """walrus in this env encodes at most 1 sync wait per instruction: hoist
    extra waits onto same-engine NOPs inserted just before the instruction.
    Safe because waits are AND-ed stall conditions on the engine's sequencer
    and semaphores are monotonic."""
    count = 0
    for fn in nc.m.functions:
        for bb in fn.blocks:
            insts = list(bb.instructions)
            new = []
            changed = False
            for inst in insts:
                si = getattr(inst, "sync_info", None)
                waits = list(si.on_wait) if (si is not None and si.on_wait) else []
                if len(waits) > 1:
                    for w in waits[:-1]:
                        nop = mybir.InstNoOp(name=f"NOPW-{count}", ins=[], outs=[])
                        count += 1
                        nop.engine = inst.engine
                        nop.sync_info = mybir.SyncInfo(on_wait=[w], on_update=[])
                        nc.register_instruction(nop, overwrite=True)
                        new.append(nop)
                    si.on_wait = [waits[-1]]
                    changed = True
                new.append(inst)
            if changed:
                bb.instructions[:] = new


def _patch_drain_wait_limit():
    if getattr(tile.TileContext, "_wait_split_patched", False):
        return
    orig = tile.TileContext.schedule_and_allocate

    def schedule_and_allocate(self, *a, **k):
        r = orig(self, *a, **k)
        _split_all_waits(self.nc)
        return r

    tile.TileContext.schedule_and_allocate = schedule_and_allocate
    tile.TileContext._wait_split_patched = True


def build_matrices():
    """S2w[p, (v*NT+t)*128 + o]: one-hot 2.0 d-sum weights; lo (v=0) maps
    tile t's kernels to output rows 0..63, hi (v=1) to 64..127.
    Sg: one-hot -1.0 (Gsneg accumulation, all tiles). I64lo/hi [64, 128]:
    G-inject selection (fp32 -> f32r on chip). Isel lo/hi [128, 64] bf16:
    k-stack half selection for the column matmuls."""
    S2w = np.zeros((128, 2 * NT * 128), dtype=ml_dtypes.bfloat16)
    Sg = np.zeros((128, NT * NK), dtype=ml_dtypes.bfloat16)
    for t in range(NT):
        for p in range(128):
            k = 8 * t + p // 16
            S2w[p, t * 128 + k] = 2.0
            S2w[p, (NT + t) * 128 + 64 + k] = 2.0
            Sg[p, t * NK + k] = -1.0
    I64w = np.zeros((64, 2 * 128), dtype=np.float32)
    I64w[:, 0:64] = np.eye(64)
    I64w[:, 128 + 64 : 256] = np.eye(64)
    Isel = np.zeros((128, 2 * NK), dtype=ml_dtypes.bfloat16)
    Isel[0:64, 0:64] = np.eye(64)
    Isel[64:128, 64:128] = np.eye(64)
    return S2w, Sg, I64w, Isel


def pack_inputs(x: np.ndarray, T: np.ndarray):
    """Host-side: transpose+roll per core, pack to [128, .] bf16 layouts."""
    xT = np.ascontiguousarray(x.T.astype(np.float32, copy=False))
    Tb = T.astype(ml_dtypes.bfloat16)  # [F, KD]
    # T_pack[p, f*KD + kd] = T[f*128 + p, kd]
    T_pack = np.ascontiguousarray(
        Tb.reshape(NF, 128, KD).transpose(1, 0, 2).reshape(128, NF * KD)
    )
    xpacks = []
    for c in range(N_CORES):
        xTc = np.roll(xT, -NI * c, axis=1)[:, :JL].astype(ml_dtypes.bfloat16)
        xpacks.append(
            np.ascontiguousarray(
                xTc.reshape(NF, 128, JL).transpose(1, 0, 2).reshape(128, NF * JL)
            )
        )
    return T_pack, xpacks


def build_program(repeat: int = 1):
    _patch_drain_wait_limit()
    nc = bass.Bass(
        "TRN2", target_bir_lowering=False, debug=False, num_devices=N_CORES
    )
    xp_d = nc.dram_tensor("xp", [128, NF * JL], _BF16, kind="ExternalInput").ap()
    Tp_d = nc.dram_tensor("Tp", [128, NF * KD], _BF16, kind="ExternalInput").ap()
    S_d = nc.dram_tensor("S", [128, 2 * NT * 128], _BF16, kind="ExternalInput").ap()
    Sg_d = nc.dram_tensor("Sg", [128, NT * NK], _BF16, kind="ExternalInput").ap()
    I64w_d = nc.dram_tensor("I64w", [NK, 2 * 128], _FP32, kind="ExternalInput").ap()
    Isel_d = nc.dram_tensor("Isel", [128, 2 * NK], _BF16, kind="ExternalInput").ap()
    orow_d = nc.dram_tensor("orow2", [128, 2 * NG], _FP32, kind="ExternalOutput").ap()
    ocol_d = nc.dram_tensor("ocol", [NK, JL], _FP32, kind="ExternalOutput").ap()

    AF = mybir.ActivationFunctionType
    AO = mybir.AluOpType

    with tile.TileContext(nc) as tc:
        with (
            tc.tile_pool(name="tin", bufs=1) as t_in,
            tc.tile_pool(name="mt", bufs=N_DVE) as mt_pool,
            tc.tile_pool(name="mtg", bufs=2) as mtg_pool,
            tc.tile_pool(name="ssb", bufs=1) as s_pool,
            tc.tile_pool(name="ab", bufs=24) as ab_pool,
            tc.tile_pool(name="ep", bufs=3) as e_pool,
            tc.tile_pool(name="op", bufs=1) as o_pool,
            # 5 rotating banks (pwarm, phase-1 pmA/pm5, pg, pgb, l1 groups)
            # + 3 resident banks (pmB0, pmB1, O_col) = 8 PSUM banks total
            tc.tile_pool(name="pp", bufs=5, space="PSUM") as psum_p,
            tc.tile_pool(name="pres", bufs=3, space="PSUM") as psum_res,
            tc.For_i(0, repeat, 1) if repeat > 1 else contextlib.nullcontext(),
        ):
            # ---- zero tiles + PE warmup (no input deps) ----
            zl = s_pool.tile([128, 128], _BF16, tag="zl")
            nc.vector.memset(zl, 0.0)
            zr = s_pool.tile([128, 512], _BF16, tag="zr")
            nc.vector.memset(zr, 0.0)
            pwarm = psum_p.tile([128, 512], _FP32, tag="pp")
            for _ in range(12):
                nc.tensor.matmul(pwarm, lhsT=zl, rhs=zr, start=True, stop=True)
            junk = s_pool.tile([128, 4], _FP32, tag="junk")
            nc.vector.tensor_copy(junk, pwarm[:, 0:4])

            # ---- loads (bf16, host-packed; few big DMAs) ----
            x_sb = t_in.tile([128, NF * JL], _BF16, tag="xsb")
            nc.sync.dma_start(out=x_sb, in_=xp_d)
            T_sb = t_in.tile([128, NF * KD], _BF16, tag="tsb")
            NQ = 4
            qf = NF // NQ  # f chunks per DMA
            for q in range(NQ):
                nc.sync.dma_start(
                    out=T_sb[:, q * qf * KD : (q + 1) * qf * KD],
                    in_=Tp_d[:, q * qf * KD : (q + 1) * qf * KD],
                )
            S_sb = s_pool.tile([128, 2 * NT * 128], _BF16, tag="s")
            nc.sync.dma_start(out=S_sb, in_=S_d)
            Sg_sb = s_pool.tile([128, NT * NK], _BF16, tag="sg")
            nc.sync.dma_start(out=Sg_sb, in_=Sg_d)
            st = s_pool.tile([NK, 2 * 128], _FP32, tag="i64stage")
            nc.sync.dma_start(out=st, in_=I64w_d)
            I64w_r = s_pool.tile([NK, 2 * 128], _F32R, tag="i64r")
            nc.vector.tensor_copy(I64w_r, st)
            Isel_sb = s_pool.tile([128, 2 * NK], _BF16, tag="isel")
            nc.sync.dma_start(out=Isel_sb, in_=Isel_d)

            def Tsl(f, t):
                return T_sb[:, f * KD + t * 128 : f * KD + (t + 1) * 128]

            def xsl(f):
                return x_sb[:, f * JL : (f + 1) * JL]

            # ---- phase 1: M^T tiles; t=0..4 f-major in rotating banks,
            # t=5 short pass, t=6..7 in resident banks (ACT reads them) ----
            col_f32 = s_pool.tile([128, N_DVE * NI], _FP32, tag="colf")
            colneg = s_pool.tile([128, 2 * NI], _FP32, tag="colneg")
            mt = [None] * NT

            def finish_dve_tile(t, pm):
                m = mt_pool.tile([128, JL], _BF16, tag="mt", name=f"mt{t}")
                if t % 2 == 0:
                    nc.vector.tensor_copy(m, pm)
                    nc.scalar.copy(col_f32[:, t * NI : (t + 1) * NI], pm[:, 0:NI])
                else:
                    nc.scalar.copy(m, pm)
                    nc.vector.tensor_copy(
                        col_f32[:, t * NI : (t + 1) * NI], pm[:, 0:NI]
                    )
                mt[t] = m

            NPA = 5
            pmA = [
                psum_p.tile([128, JL], _FP32, tag="pp", name=f"pmA{t}")
                for t in range(NPA)
            ]
            for q in range(NQ):
                for f in range(q * qf, (q + 1) * qf):
                    for t in range(NPA):
                        nc.tensor.matmul(
                            pmA[t],
                            lhsT=Tsl(f, t),
                            rhs=xsl(f),
                            start=(f == 0),
                            stop=(f == NF - 1),
                        )
            for t in range(NPA):
                finish_dve_tile(t, pmA[t])
            # pass B: tile 5 (rotating) + tiles 6,7 (resident, ACT source)
            pm5 = psum_p.tile([128, JL], _FP32, tag="pp")
            pmB = [
                psum_res.tile([128, JL], _FP32, tag="pres", name=f"pmB{t}")
                for t in range(2)
            ]
            for f in range(NF):
                nc.tensor.matmul(
                    pm5, lhsT=Tsl(f, 5), rhs=xsl(f),
                    start=(f == 0), stop=(f == NF - 1),
                )
                for t in range(2):
                    nc.tensor.matmul(
                        pmB[t],
                        lhsT=Tsl(f, N_DVE + t),
                        rhs=xsl(f),
                        start=(f == 0),
                        stop=(f == NF - 1),
                    )
            finish_dve_tile(5, pm5)
            for t in range(2):
                m = mtg_pool.tile([128, JL], _BF16, tag="mtg", name=f"mtg{t}")
                nc.scalar.copy(m, pmB[t])
                mt[N_DVE + t] = m
                # colneg = -M^T[:, 0:NI] (ACT relu bias)
                nc.vector.tensor_scalar(
                    colneg[:, t * NI : (t + 1) * NI],
                    pmB[t][:, 0:NI],
                    -1.0,
                    None,
                    op0=AO.mult,
                )

            # ---- phase 1.5: Gsneg = -G; Gb (k-stacked exp bias) ----
            pg = psum_p.tile([NK, JL], _FP32, tag="pp")
            for t in range(NT):
                nc.tensor.matmul(
                    pg,
                    lhsT=Sg_sb[:, t * NK : (t + 1) * NK],
                    rhs=mt[t],
                    start=(t == 0),
                    stop=(t == NT - 1),
                )
            Gs_r = s_pool.tile([NK, JL], _F32R, tag="gsr")
            nc.vector.tensor_copy(Gs_r, pg)
            # Gb[p, h*NG+g] = Gsneg[p, 4g+h] (p<64) / Gsneg[p-64, 4g+2+h]
            pgb = psum_p.tile([128, 2 * NG], _FP32, tag="pp")
            for h in range(2):
                for v in range(2):
                    nc.tensor.matmul(
                        pgb[:, h * NG : (h + 1) * NG],
                        lhsT=I64w_r[:, v * 128 : (v + 1) * 128],
                        rhs=Gs_r[:, 2 * v + h : 2 * v + h + 4 * (NG - 1) + 1 : 4],
                        start=(v == 0),
                        stop=(v == 1),
                    )
            Gb = s_pool.tile([128, 2 * NG], _FP32, tag="gb")
            nc.vector.tensor_copy(Gb, pgb)

            # ---- phase 2 ----
            O_row = o_pool.tile([128, 2 * NG], _FP32, tag="orow")
            O_colp = psum_res.tile([NK, JL], _FP32, tag="pres")
            nc.tensor.matmul(
                O_colp, lhsT=zl[0:1, 0:NK], rhs=zr[0:1, 0:JL], start=True, stop=False
            )

            for g in range(NG):
                rows = [4 * g, 4 * g + 1, 4 * g + 2, 4 * g + 3]
                # relu tiles: ab[v][t] is [128, 512] (h=0/1 halves)
                ab = [[None] * NT, [None] * NT]
                for t in range(NT):
                    for v in range(2):
                        a = ab_pool.tile(
                            [128, 2 * W], _BF16, tag="ab", name=f"ab{g}_{t}_{v}"
                        )
                        for h in range(2):
                            i = rows[2 * v + h]
                            dst = a[:, h * W : (h + 1) * W]
                            if t < N_DVE:
                                nc.vector.tensor_scalar(
                                    dst,
                                    mt[t][:, i + 1 : i + 1 + W],
                                    col_f32[:, t * NI + i : t * NI + i + 1],
                                    0.0,
                                    op0=AO.subtract,
                                    op1=AO.max,
                                )
                            else:
                                tb = t - N_DVE
                                nc.scalar.activation(
                                    dst,
                                    pmB[tb][:, i + 1 : i + 1 + W],
                                    AF.Relu,
                                    bias=colneg[:, tb * NI + i : tb * NI + i + 1],
                                    scale=1.0,
                                )
                        ab[v][t] = a
                l1 = psum_p.tile([128, 2 * W], _FP32, tag="pp", name=f"l1_{g}")
                for t in range(NT):
                    for v in range(2):
                        nc.tensor.matmul(
                            l1,
                            lhsT=S_sb[:, (v * NT + t) * 128 : (v * NT + t + 1) * 128],
                            rhs=ab[v][t],
                            start=(t == 0 and v == 0),
                            stop=False,
                        )
                for h in range(2):
                    for v in range(2):
                        i = rows[2 * v + h]
                        nc.tensor.matmul(
                            l1[:, h * W : (h + 1) * W],
                            lhsT=I64w_r[:, v * 128 : (v + 1) * 128],
                            rhs=Gs_r[:, i + 1 : i + 1 + W],
                            start=False,
                            stop=(h == 1 and v == 1),
                        )
                E = e_pool.tile([128, 2 * W], _BF16, tag="ep")
                for h in range(2):
                    c = h * NG + g
                    nc.scalar.activation(
                        E[:, h * W : (h + 1) * W],
                        l1[:, h * W : (h + 1) * W],
                        AF.Exp,
                        scale=-1.0,
                        bias=Gb[:, c : c + 1],
                        accum_out=O_row[:, c : c + 1],
                    )
                for h in range(2):
                    for v in range(2):
                        i = rows[2 * v + h]
                        nc.tensor.matmul(
                            O_colp[:, i + 1 : i + W],
                            lhsT=Isel_sb[:, v * NK : (v + 1) * NK],
                            rhs=E[:, h * W : h * W + W - 1],
                            start=False,
                            stop=(g == NG - 1 and h == 1 and v == 1),
                        )

            O_col = o_pool.tile([NK, JL], _FP32, tag="ocol")
            nc.scalar.copy(O_col, O_colp)
            nc.sync.dma_start(out=orow_d, in_=O_row)
            nc.sync.dma_start(out=ocol_d, in_=O_col)
    return nc


_CACHED = {}


def _get_program(repeat: int = 1):
    key = f"nc{repeat}"
    if key not in _CACHED:
        _CACHED[key] = build_program(repeat)
        _CACHED["S"] = build_matrices()
    return _CACHED[key], _CACHED["S"]


def make_in_maps(x: np.ndarray, T: np.ndarray, S2w, Sg, I64w, Isel):
    T_pack, xpacks = pack_inputs(x, T)
    in_maps = []
    for c in range(N_CORES):
        in_maps.append(
            {
                "xp": xpacks[c],
                "Tp": T_pack,
                "S": S2w,
                "Sg": Sg,
                "I64w": I64w,
                "Isel": Isel,
            }
        )
    return in_maps


def assemble(results) -> np.ndarray:
    out = np.zeros((B, NK), dtype=np.float64)
    for c in range(N_CORES):
        R = results[c]["orow2"]  # [128, 2*NG] k-stacked row sums
        C = results[c]["ocol"]  # [k, j_local]
        for col in range(2 * NG):
            h, g = divmod(col, NG)
            rA = 4 * g + h
            rB = 4 * g + 2 + h
            out[NI * c + rA, :] += R[0:NK, col]
            out[NI * c + rB, :] += R[NK:128, col]
        Cfull = np.zeros((B, NK), dtype=np.float64)
        Cfull[:JL] = C.T
        out += np.roll(Cfull, NI * c, axis=0)
    return out.astype(np.float32)


def run(
    x: np.ndarray,
    T: np.ndarray,
    trace: bool = False,
    repeat: int = 1,
    tmpdir: str | None = None,
):
    nc, mats = _get_program(repeat)
    in_maps = make_in_maps(x, T, *mats)
    res = bass_utils.run_bass_kernel_spmd(
        nc, in_maps, core_ids=list(range(N_CORES)), trace=trace, tmpdir=tmpdir
    )
    return assemble(res.results), res


def kernel(x: np.ndarray, T: np.ndarray) -> np.ndarray:
    out, _ = run(x, T)
    return out


# revision 16
# speedup vs baseline: 1.0788x; 1.0319x over previous
"""MinibatchDiscrimination TRN2 kernel (v4).

x: [512, 1024] f32, T: [1024, 1024] f32.
M = (x @ T).reshape(512, 64, 16); l1[i,j,k] = sum_d |M[i,kd]-M[j,kd]|
out[i,k] = sum_j exp(-l1[i,j,k]) - 1.

Sharding: batch rows split across 8 cores (64 each), no collectives. Each
core's copy of x^T has its j-columns rolled so its own 64 rows sit at
local columns 0..63. Symmetric pair coverage as v1-v3: row i covers the
j-window [i+1, i+257); d=1..255 pairs counted once (row-side via exp
accum, col-side via a PSUM column accumulator on the PE), d=256 twice
(row-side only); the diagonal is never computed.

Measured op costs (this silicon): DVE tensor_scalar = 58 + FD/2 cyc
(2x max — no 4x uop for TS, any dtype); ACT activation = ~(init + FD)
cyc at 1.2GHz with init 222 (SBUF) / 172 (PSUM src) + 284ns for the
accumulator read; GPSIMD elementwise is ~4us/tile (useless); PE bf16
matmul = 1 col/cyc at 2.4GHz once the HAM clock gate is warm.

v4 structure:
- Uniform s=+1 relu identity for ALL kd tiles:
    sum_d |z_d| = 2*sum_d relu(z_d) - (G_j - G_i),  G_j = sum_d M[j,kd]
  DVE computes relu via (z sub col) max 0 (tiles 0..5, bf16 in/out,
  ~196ns); ACT computes relu via Relu(in + bias=-col) reading M^T
  directly from resident PSUM banks (tiles 6..7, ~357ns). -G_j is
  injected into the l1 PSUM by I64-style matmuls; +G_i rides exp bias.
- 2-pair k-stacked groups: 4 rows per group. l1 PSUM is [128, 512]:
  partitions 0..63 = pair A's 64 kernels, 64..127 = pair B's. The S
  matmul lhsT is [128, 128] (lo variant fills output rows 0..63 from
  pair A's relu tile, hi variant rows 64..127 from pair B's). One exp
  instruction [128, 256] then covers TWO rows (halves ACT exp+accum
  cost/row); its bias/accum use k-stacked Gb / O_row2 [128, .] built
  with selection matmuls.
- Column accumulation on the PE: rhs = full E [128, 255], lhsT =
  selection matrix Isel_v [128, 64] picking pair v's half; accumulates
  into a persistent PSUM O_col bank (zeroed once by a rank-1 matmul).
- bf16 host-packed loads: T/x are shipped pre-transposed as [128, .]
  bf16 so phase 1 needs no staging or cast; phase-1 matmuls are bf16.
- PE warmup: 12 dummy FD=512 matmuls during the DMA window so the HAM
  clock gate reaches 2.4GHz before phase 1.
- Phase 1 f-major with 6 live PSUM banks (tiles 0..5) as chunks
  stream in; tiles 6..7 in a short pass B stay PSUM-resident for ACT.

Engine budget per pair (measured rates): PE 8x512+2x256+2x255 cyc
~2.3us/pair-equivalent, DVE 12 relu x 196ns = 2.35us, ACT 4 relu x
357 + exp-side 744 = 2.17us.
"""

import contextlib

import numpy as np
import ml_dtypes

import concourse.bass as bass
import concourse.tile as tile
from concourse import mybir
from concourse import bass_utils

B = 512
F = 1024
KD = 1024  # = NUM_KERNELS(64) * KERNEL_DIM(16)
NK = 64
N_CORES = 8
NI = B // N_CORES  # local rows per core
NT = KD // 128  # kd tiles
NF = F // 128  # f chunks
W = 256  # j-window width per row
JL = NI + W  # used local-j extent
NG = NI // 4  # 2-pair groups (4 rows each)
N_DVE = 6  # tiles 0..5 on DVE, 6..7 on ACT (from resident PSUM)

_FP32 = mybir.dt.float32
_F32R = mybir.dt.float32r
_BF16 = mybir.dt.bfloat16


def _split_all_waits(nc):
    """walrus in this env encodes at most 1 sync wait per instruction: hoist
    extra waits onto same-engine NOPs inserted just before the instruction.
    Safe because waits are AND-ed stall conditions on the engine's sequencer
    and semaphores are monotonic."""
    count = 0
    for fn in nc.m.functions:
        for bb in fn.blocks:
            insts = list(bb.instructions)
            new = []
            changed = False
            for inst in insts:
                si = getattr(inst, "sync_info", None)
                waits = list(si.on_wait) if (si is not None and si.on_wait) else []
                if len(waits) > 1:
                    for w in waits[:-1]:
                        nop = mybir.InstNoOp(name=f"NOPW-{count}", ins=[], outs=[])
                        count += 1
                        nop.engine = inst.engine
                        nop.sync_info = mybir.SyncInfo(on_wait=[w], on_update=[])
                        nc.register_instruction(nop, overwrite=True)
                        new.append(nop)
                    si.on_wait = [waits[-1]]
                    changed = True
                new.append(inst)
            if changed:
                bb.instructions[:] = new


def _patch_drain_wait_limit():
    if getattr(tile.TileContext, "_wait_split_patched", False):
        return
    orig = tile.TileContext.schedule_and_allocate

    def schedule_and_allocate(self, *a, **k):
        r = orig(self, *a, **k)
        _split_all_waits(self.nc)
        return r

    tile.TileContext.schedule_and_allocate = schedule_and_allocate
    tile.TileContext._wait_split_patched = True


def build_matrices():
    """S2w[p, (v*NT+t)*128 + o]: one-hot 2.0 d-sum weights; lo (v=0) maps
    tile t's kernels to output rows 0..63, hi (v=1) to 64..127.
    Sg: one-hot -1.0 (Gsneg accumulation, all tiles). I64lo/hi [64, 128]:
    G-inject selection (fp32 -> f32r on chip). Isel lo/hi [128, 64] bf16:
    k-stack half selection for the column matmuls."""
    S2w = np.zeros((128, 2 * NT * 128), dtype=ml_dtypes.bfloat16)
    Sg = np.zeros((128, NT * NK), dtype=ml_dtypes.bfloat16)
    for t in range(NT):
        for p in range(128):
            k = 8 * t + p // 16
            S2w[p, t * 128 + k] = 2.0
            S2w[p, (NT + t) * 128 + 64 + k] = 2.0
            Sg[p, t * NK + k] = -1.0
    I64w = np.zeros((64, 2 * 128), dtype=np.float32)
    I64w[:, 0:64] = np.eye(64)
    I64w[:, 128 + 64 : 256] = np.eye(64)
    Isel = np.zeros((128, 2 * NK), dtype=ml_dtypes.bfloat16)
    Isel[0:64, 0:64] = np.eye(64)
    Isel[64:128, 64:128] = np.eye(64)
    return S2w, Sg, I64w, Isel


def pack_inputs(x: np.ndarray, T: np.ndarray):
    """Host-side: transpose+roll per core, pack to [128, .] bf16 layouts."""
    xT = np.ascontiguousarray(x.T.astype(np.float32, copy=False))
    Tb = T.astype(ml_dtypes.bfloat16)  # [F, KD]
    # T_pack[p, f*KD + kd] = T[f*128 + p, kd]
    T_pack = np.ascontiguousarray(
        Tb.reshape(NF, 128, KD).transpose(1, 0, 2).reshape(128, NF * KD)
    )
    xpacks = []
    for c in range(N_CORES):
        xTc = np.roll(xT, -NI * c, axis=1)[:, :JL].astype(ml_dtypes.bfloat16)
        xpacks.append(
            np.ascontiguousarray(
                xTc.reshape(NF, 128, JL).transpose(1, 0, 2).reshape(128, NF * JL)
            )
        )
    return T_pack, xpacks


def build_program(repeat: int = 1):
    _patch_drain_wait_limit()
    nc = bass.Bass(
        "TRN2", target_bir_lowering=False, debug=False, num_devices=N_CORES
    )
    xp_d = nc.dram_tensor("xp", [128, NF * JL], _BF16, kind="ExternalInput").ap()
    Tp_d = nc.dram_tensor("Tp", [128, NF * KD], _BF16, kind="ExternalInput").ap()
    S_d = nc.dram_tensor("S", [128, 2 * NT * 128], _BF16, kind="ExternalInput").ap()
    Sg_d = nc.dram_tensor("Sg", [128, NT * NK], _BF16, kind="ExternalInput").ap()
    I64w_d = nc.dram_tensor("I64w", [NK, 2 * 128], _FP32, kind="ExternalInput").ap()
    Isel_d = nc.dram_tensor("Isel", [128, 2 * NK], _BF16, kind="ExternalInput").ap()
    orow_d = nc.dram_tensor("orow2", [128, 2 * NG], _FP32, kind="ExternalOutput").ap()
    ocol_d = nc.dram_tensor("ocol", [NK, JL], _FP32, kind="ExternalOutput").ap()

    AF = mybir.ActivationFunctionType
    AO = mybir.AluOpType

    with tile.TileContext(nc) as tc:
        with (
            tc.tile_pool(name="tin", bufs=1) as t_in,
            tc.tile_pool(name="mt", bufs=N_DVE) as mt_pool,
            tc.tile_pool(name="mtg", bufs=2) as mtg_pool,
            tc.tile_pool(name="ssb", bufs=1) as s_pool,
            tc.tile_pool(name="ab", bufs=24) as ab_pool,
            tc.tile_pool(name="ep", bufs=3) as e_pool,
            tc.tile_pool(name="op", bufs=1) as o_pool,
            # 5 rotating banks (pwarm, phase-1 pmA/pm5, pg, pgb, l1 groups)
            # + 3 resident banks (pmB0, pmB1, O_col) = 8 PSUM banks total
            tc.tile_pool(name="pp", bufs=5, space="PSUM") as psum_p,
            tc.tile_pool(name="pres", bufs=3, space="PSUM") as psum_res,
            tc.For_i(0, repeat, 1) if repeat > 1 else contextlib.nullcontext(),
        ):
            # ---- zero tiles + PE warmup (no input deps) ----
            zl = s_pool.tile([128, 128], _BF16, tag="zl")
            nc.vector.memset(zl, 0.0)
            zr = s_pool.tile([128, 512], _BF16, tag="zr")
            nc.vector.memset(zr, 0.0)
            pwarm = psum_p.tile([128, 512], _FP32, tag="pp")
            for _ in range(12):
                nc.tensor.matmul(pwarm, lhsT=zl, rhs=zr, start=True, stop=True)
            junk = s_pool.tile([128, 4], _FP32, tag="junk")
            nc.vector.tensor_copy(junk, pwarm[:, 0:4])

            # ---- loads (bf16, host-packed; split over SP+ACT DMA queues) ----
            x_sb = t_in.tile([128, NF * JL], _BF16, tag="xsb")
            nc.sync.dma_start(out=x_sb, in_=xp_d)
            T_sb = t_in.tile([128, NF * KD], _BF16, tag="tsb")
            NQ = 4
            qf = NF // NQ  # f chunks per DMA
            for q in range(NQ):
                eng = nc.sync if q % 2 == 0 else nc.scalar
                eng.dma_start(
                    out=T_sb[:, q * qf * KD : (q + 1) * qf * KD],
                    in_=Tp_d[:, q * qf * KD : (q + 1) * qf * KD],
                )
            S_sb = s_pool.tile([128, 2 * NT * 128], _BF16, tag="s")
            nc.scalar.dma_start(out=S_sb, in_=S_d)
            Sg_sb = s_pool.tile([128, NT * NK], _BF16, tag="sg")
            nc.sync.dma_start(out=Sg_sb, in_=Sg_d)
            st = s_pool.tile([NK, 2 * 128], _FP32, tag="i64stage")
            nc.scalar.dma_start(out=st, in_=I64w_d)
            I64w_r = s_pool.tile([NK, 2 * 128], _F32R, tag="i64r")
            nc.vector.tensor_copy(I64w_r, st)
            Isel_sb = s_pool.tile([128, 2 * NK], _BF16, tag="isel")
            nc.sync.dma_start(out=Isel_sb, in_=Isel_d)

            def Tsl(f, t):
                return T_sb[:, f * KD + t * 128 : f * KD + (t + 1) * 128]

            def xsl(f):
                return x_sb[:, f * JL : (f + 1) * JL]

            # ---- phase 1: M^T tiles; t=0..4 f-major in rotating banks,
            # t=5 short pass, t=6..7 in resident banks (ACT reads them) ----
            col_f32 = s_pool.tile([128, (N_DVE + 1) * NI], _FP32, tag="colf")
            colneg = s_pool.tile([128, 2 * NI], _FP32, tag="colneg")
            mt = [None] * NT

            def finish_dve_tile(t, pm):
                m = mt_pool.tile([128, JL], _BF16, tag="mt", name=f"mt{t}")
                if t % 2 == 0:
                    nc.vector.tensor_copy(m, pm)
                    nc.scalar.copy(col_f32[:, t * NI : (t + 1) * NI], pm[:, 0:NI])
                else:
                    nc.scalar.copy(m, pm)
                    nc.vector.tensor_copy(
                        col_f32[:, t * NI : (t + 1) * NI], pm[:, 0:NI]
                    )
                mt[t] = m

            NPA = 5
            pmA = [
                psum_p.tile([128, JL], _FP32, tag="pp", name=f"pmA{t}")
                for t in range(NPA)
            ]
            for q in range(NQ):
                for f in range(q * qf, (q + 1) * qf):
                    for t in range(NPA):
                        nc.tensor.matmul(
                            pmA[t],
                            lhsT=Tsl(f, t),
                            rhs=xsl(f),
                            start=(f == 0),
                            stop=(f == NF - 1),
                        )
            for t in range(NPA):
                finish_dve_tile(t, pmA[t])
            # pass B: tile 5 (rotating) + tiles 6,7 (resident, ACT source)
            pm5 = psum_p.tile([128, JL], _FP32, tag="pp")
            pmB = [
                psum_res.tile([128, JL], _FP32, tag="pres", name=f"pmB{t}")
                for t in range(2)
            ]
            for f in range(NF):
                nc.tensor.matmul(
                    pm5, lhsT=Tsl(f, 5), rhs=xsl(f),
                    start=(f == 0), stop=(f == NF - 1),
                )
                for t in range(2):
                    nc.tensor.matmul(
                        pmB[t],
                        lhsT=Tsl(f, N_DVE + t),
                        rhs=xsl(f),
                        start=(f == 0),
                        stop=(f == NF - 1),
                    )
            finish_dve_tile(5, pm5)
            for t in range(2):
                m = mtg_pool.tile([128, JL], _BF16, tag="mtg", name=f"mtg{t}")
                nc.scalar.copy(m, pmB[t])
                mt[N_DVE + t] = m
                # colneg = -M^T[:, 0:NI] (ACT relu bias)
                nc.vector.tensor_scalar(
                    colneg[:, t * NI : (t + 1) * NI],
                    pmB[t][:, 0:NI],
                    -1.0,
                    None,
                    op0=AO.mult,
                )
            # positive col for tile 6 (one of its halves runs on DVE)
            nc.scalar.copy(col_f32[:, N_DVE * NI : (N_DVE + 1) * NI], pmB[0][:, 0:NI])

            # ---- phase 1.5: Gsneg = -G; Gb (k-stacked exp bias) ----
            pg = psum_p.tile([NK, JL], _FP32, tag="pp")
            for t in range(NT):
                nc.tensor.matmul(
                    pg,
                    lhsT=Sg_sb[:, t * NK : (t + 1) * NK],
                    rhs=mt[t],
                    start=(t == 0),
                    stop=(t == NT - 1),
                )
            Gs_r = s_pool.tile([NK, JL], _F32R, tag="gsr")
            nc.vector.tensor_copy(Gs_r, pg)
            # Gb[p, h*NG+g] = Gsneg[p, 4g+h] (p<64) / Gsneg[p-64, 4g+2+h]
            pgb = psum_p.tile([128, 2 * NG], _FP32, tag="pp")
            for h in range(2):
                for v in range(2):
                    nc.tensor.matmul(
                        pgb[:, h * NG : (h + 1) * NG],
                        lhsT=I64w_r[:, v * 128 : (v + 1) * 128],
                        rhs=Gs_r[:, 2 * v + h : 2 * v + h + 4 * (NG - 1) + 1 : 4],
                        start=(v == 0),
                        stop=(v == 1),
                    )
            Gb = s_pool.tile([128, 2 * NG], _FP32, tag="gb")
            nc.vector.tensor_copy(Gb, pgb)

            # ---- phase 2 ----
            O_row = o_pool.tile([128, 2 * NG], _FP32, tag="orow")
            O_colp = psum_res.tile([NK, JL], _FP32, tag="pres")
            nc.tensor.matmul(
                O_colp, lhsT=zl[0:1, 0:NK], rhs=zr[0:1, 0:JL], start=True, stop=False
            )

            for g in range(NG):
                rows = [4 * g, 4 * g + 1, 4 * g + 2, 4 * g + 3]
                # relu tiles: ab[v][t] is [128, 512] (h=0/1 halves)
                ab = [[None] * NT, [None] * NT]
                for t in range(NT):
                    for v in range(2):
                        a = ab_pool.tile(
                            [128, 2 * W], _BF16, tag="ab", name=f"ab{g}_{t}_{v}"
                        )
                        for h in range(2):
                            i = rows[2 * v + h]
                            dst = a[:, h * W : (h + 1) * W]
                            # 25 DVE / 7 ACT halves per group (measured
                            # rates: DVE 196ns vs ACT 474ns per half)
                            on_dve = t < N_DVE or (t == N_DVE and v == 0 and h == 0)
                            if on_dve:
                                nc.vector.tensor_scalar(
                                    dst,
                                    mt[t][:, i + 1 : i + 1 + W],
                                    col_f32[:, t * NI + i : t * NI + i + 1],
                                    0.0,
                                    op0=AO.subtract,
                                    op1=AO.max,
                                )
                            else:
                                tb = t - N_DVE
                                nc.scalar.activation(
                                    dst,
                                    pmB[tb][:, i + 1 : i + 1 + W],
                                    AF.Relu,
                                    bias=colneg[:, tb * NI + i : tb * NI + i + 1],
                                    scale=1.0,
                                )
                        ab[v][t] = a
                l1 = psum_p.tile([128, 2 * W], _FP32, tag="pp", name=f"l1_{g}")
                for t in range(NT):
                    for v in range(2):
                        nc.tensor.matmul(
                            l1,
                            lhsT=S_sb[:, (v * NT + t) * 128 : (v * NT + t + 1) * 128],
                            rhs=ab[v][t],
                            start=(t == 0 and v == 0),
                            stop=False,
                        )
                for h in range(2):
                    for v in range(2):
                        i = rows[2 * v + h]
                        nc.tensor.matmul(
                            l1[:, h * W : (h + 1) * W],
                            lhsT=I64w_r[:, v * 128 : (v + 1) * 128],
                            rhs=Gs_r[:, i + 1 : i + 1 + W],
                            start=False,
                            stop=(h == 1 and v == 1),
                        )
                E = e_pool.tile([128, 2 * W], _BF16, tag="ep")
                for h in range(2):
                    c = h * NG + g
                    nc.scalar.activation(
                        E[:, h * W : (h + 1) * W],
                        l1[:, h * W : (h + 1) * W],
                        AF.Exp,
                        scale=-1.0,
                        bias=Gb[:, c : c + 1],
                        accum_out=O_row[:, c : c + 1],
                    )
                for h in range(2):
                    for v in range(2):
                        i = rows[2 * v + h]
                        nc.tensor.matmul(
                            O_colp[:, i + 1 : i + W],
                            lhsT=Isel_sb[:, v * NK : (v + 1) * NK],
                            rhs=E[:, h * W : h * W + W - 1],
                            start=False,
                            stop=(g == NG - 1 and h == 1 and v == 1),
                        )

            O_col = o_pool.tile([NK, JL], _FP32, tag="ocol")
            nc.scalar.copy(O_col, O_colp)
            nc.sync.dma_start(out=orow_d, in_=O_row)
            nc.sync.dma_start(out=ocol_d, in_=O_col)
    return nc


_CACHED = {}


def _get_program(repeat: int = 1):
    key = f"nc{repeat}"
    if key not in _CACHED:
        _CACHED[key] = build_program(repeat)
        _CACHED["S"] = build_matrices()
    return _CACHED[key], _CACHED["S"]


def make_in_maps(x: np.ndarray, T: np.ndarray, S2w, Sg, I64w, Isel):
    T_pack, xpacks = pack_inputs(x, T)
    in_maps = []
    for c in range(N_CORES):
        in_maps.append(
            {
                "xp": xpacks[c],
                "Tp": T_pack,
                "S": S2w,
                "Sg": Sg,
                "I64w": I64w,
                "Isel": Isel,
            }
        )
    return in_maps


def assemble(results) -> np.ndarray:
    out = np.zeros((B, NK), dtype=np.float64)
    for c in range(N_CORES):
        R = results[c]["orow2"]  # [128, 2*NG] k-stacked row sums
        C = results[c]["ocol"]  # [k, j_local]
        for col in range(2 * NG):
            h, g = divmod(col, NG)
            rA = 4 * g + h
            rB = 4 * g + 2 + h
            out[NI * c + rA, :] += R[0:NK, col]
            out[NI * c + rB, :] += R[NK:128, col]
        Cfull = np.zeros((B, NK), dtype=np.float64)
        Cfull[:JL] = C.T
        out += np.roll(Cfull, NI * c, axis=0)
    return out.astype(np.float32)


def run(
    x: np.ndarray,
    T: np.ndarray,
    trace: bool = False,
    repeat: int = 1,
    tmpdir: str | None = None,
):
    nc, mats = _get_program(repeat)
    in_maps = make_in_maps(x, T, *mats)
    res = bass_utils.run_bass_kernel_spmd(
        nc, in_maps, core_ids=list(range(N_CORES)), trace=trace, tmpdir=tmpdir
    )
    return assemble(res.results), res


def kernel(x: np.ndarray, T: np.ndarray) -> np.ndarray:
    out, _ = run(x, T)
    return out
